# revision 1
# baseline (speedup 1.0000x reference)
"""Trainium2 8-core kernel for nn_Attention_27530740367526.

Multi-head causal attention (B=2, S=2048, D=2048, H=16, HD=128, fp32) with
RoPE, sharded batch x head-group across 8 NeuronCores: core c handles batch
c//4 and heads [4*(c%4), 4*(c%4)+4).  Each core computes q/k/v projections
(+RoPE), attention for its heads, and the slice of the wo projection those
heads feed — a partial [S, D] output.  The host sums the 4 partials per
batch (the row-parallel wo "all-reduce" is a host-side unshard).

On-device everything lives in "transposed land": qT/kT are [head_dim, seq]
with head-dim on partitions, so scores come out transposed ([k, q]), the
softmax denominator is an all-ones-column matmul (giving a partition-
broadcast denominator for free), and PV / wo consume natural layouts with
zero on-device transposes.  RoPE's rotate-half is a 128x128 permutation
matmul on the PE.  All matmul operands are float32r (fp32 rounded to 11
explicit mantissa bits, pre-rounded bit-exactly on the host) which runs at
full PE rate.

The kernel is fully fused: heads are processed in pairs (so all weights +
k/v stay in SBUF), and within a pair the work is streamed per 512-column
sequence chunk — project q/k/v for the chunk, run causal attention for
that query chunk against all earlier k/v chunks (available because
processing is in causal order), and emit the previous chunk's slice of the
wo projection as PE filler work inside the attention's softmax-wait
bubbles.  The second head-pair accumulates into the output via CCE
DMA-add.
"""

import sys

if "/opt/trn_rl_repo" not in sys.path:
    sys.path.insert(0, "/opt/trn_rl_repo")

from collections import deque

import numpy as np

import concourse.bacc as bacc
import concourse.mybir as mybir
import concourse.tile as tile
from concourse.bass_utils import run_bass_kernel_spmd

F32 = mybir.dt.float32
F32R = mybir.dt.float32r
AF = mybir.ActivationFunctionType

N_HEADS = 16
N_CORES = 8
B, S, D = 2, 2048, 2048
HD = D // N_HEADS
H_LOC = N_HEADS // (N_CORES // B)  # 4 heads per core
HW = H_LOC * HD                    # 512 q/k/v columns per core
SC = 512                           # seq chunk (matmul moving free dim)
P = 128
LOOKAHEAD = 3                      # scores-tile software pipeline depth


def _round_f32r(x: np.ndarray) -> np.ndarray:
    """Host-side fp32 -> float32r rounding (RNE to 11 explicit mantissa
    bits); bit-exact with the device DVE rounding."""
    xi = np.ascontiguousarray(x, dtype=np.float32).view(np.uint32)
    nbits = 12
    lo = np.uint32((1 << nbits) - 1)
    half = np.uint32(1 << (nbits - 1))
    rem = xi & lo
    up = (rem > half) | ((rem == half) & (((xi >> nbits) & 1) == 1))
    r = (xi & ~lo) + np.where(up, np.uint32(1 << nbits), np.uint32(0))
    return r.view(np.float32)


def _build_core_kernel(causal: bool):
    KO = D // P            # 16 contraction subtiles for projections
    NQC = S // SC          # 4 q-chunks
    NSUB = SC // P         # 4 128-blocks per chunk
    NST = S // P           # 16 s-tiles
    NHB = H_LOC // 2       # head pairs
    inv_sqrt_hd = 1.0 / float(np.sqrt(HD))

    nc = bacc.Bacc(None, target_bir_lowering=False)

    xT = nc.dram_tensor("xT", [D, S], F32R, kind="ExternalInput")
    wqkvT = nc.dram_tensor(
        "wqkvT", [H_LOC // 2, D, 6 * HD], F32R, kind="ExternalInput"
    )
    woT = nc.dram_tensor("woT", [HW, D], F32R, kind="ExternalInput")
    cosT = nc.dram_tensor("cosT", [HD, S], F32, kind="ExternalInput")
    sinT = nc.dram_tensor("sinT", [HD, S], F32, kind="ExternalInput")
    PT = nc.dram_tensor("PT", [HD, HD], F32R, kind="ExternalInput")
    ones = nc.dram_tensor("ones", [P, P], F32R, kind="ExternalInput")
    if causal:
        maskT = nc.dram_tensor("maskT", [SC, SC], F32, kind="ExternalInput")
    else:
        maskT = nc.dram_tensor("maskT", [S, S], F32, kind="ExternalInput")
    y = nc.dram_tensor("y", [S, D], F32, kind="ExternalOutput")

    xT_r = xT.rearrange("(ko ki) s -> ki ko s", ki=P)
    wqkvT_r = wqkvT.rearrange("hb (ko ki) c -> hb ki ko c", ki=P)
    woT_r = woT.rearrange("(h ki) d -> ki h d", ki=P)

    with tile.TileContext(nc) as tc:
        with (
            tc.tile_pool(name="persist", bufs=1) as persist,
            tc.tile_pool(name="wpool", bufs=1) as wpool,
            tc.tile_pool(name="kvq", bufs=1) as kvq,
            tc.tile_pool(name="xa", bufs=1) as xa,
            tc.tile_pool(name="cs", bufs=2) as cspool,
            tc.tile_pool(name="scr", bufs=2) as scr,
            tc.tile_pool(name="exps", bufs=4) as expp,
            tc.tile_pool(name="outq", bufs=2) as outqp,
            tc.tile_pool(name="yo", bufs=4) as yop,
            tc.tile_pool(name="gm", bufs=3) as gmp,
            tc.tile_pool(name="ps", bufs=3, space="PSUM") as cyc,
            tc.tile_pool(name="ops", bufs=2, space="PSUM") as ops,
            tc.tile_pool(name="dps", bufs=2, space="PSUM") as dps,
            tc.tile_pool(name="yps", bufs=1, space="PSUM") as yps,
        ):
            pt_sb = persist.tile([P, HD], F32R)
            nc.sync.dma_start(pt_sb[:], PT[:])
            ones_sb = persist.tile([P, P], F32R)
            nc.sync.dma_start(ones_sb[:], ones[:])
            if causal:
                mask_sb = persist.tile([P, NSUB, SC], F32)
                nc.sync.dma_start(
                    mask_sb[:], maskT.rearrange("(j ki) q -> ki j q", ki=P)
                )

            def load_chunk(sc):
                ssl = slice(sc * SC, (sc + 1) * SC)
                xt = xa.tile([P, KO, SC], F32R, tag="xt")
                for ko in range(KO):
                    nc.sync.dma_start(xt[:, ko], xT_r[:, ko, ssl])
                cos_t = cspool.tile([P, SC], F32, tag="cos")
                sin_t = cspool.tile([P, SC], F32, tag="sin")
                nc.sync.dma_start(cos_t[:], cosT[:, ssl])
                nc.sync.dma_start(sin_t[:], sinT[:, ssl])
                return xt, cos_t, sin_t

            preloaded = None
            for hb in range(NHB):
                if hb == 0:
                    # queue the first x-chunk's DMAs ahead of the (large)
                    # weight loads so the PE can start within a few us
                    preloaded = load_chunk(0)
                # ---- weights for this head pair, one DMA per ko slice
                # (host pre-packs the pair's q/k/v columns contiguously so
                #  every descriptor is a 3 KB row)
                w_sb = wpool.tile([P, KO, 6 * HD], F32R, tag="w")
                for ko in range(KO):
                    nc.sync.dma_start(
                        w_sb[:, ko, :], wqkvT_r[hb, :, ko, :]
                    )
                wo_sb = wpool.tile([P, 2, D], F32R, tag="wo")
                for hl in range(2):
                    nc.sync.dma_start(
                        wo_sb[:, hl], woT_r[:, hb * 2 + hl, :]
                    )

                # ---- per-pair persistent qkv ----
                kT_sb = kvq.tile([P, 2, S], F32R, tag="kT")
                v_sb = kvq.tile([P, NST, 2 * HD], F32R, tag="v")
                deferred = not causal
                qT_full = (
                    kvq.tile([P, 2, S], F32R, tag="qT", name="qT_full") if deferred else None
                )

                def project_chunk(sc, loaded):
                    if deferred:
                        qT_dst = qT_full
                    else:
                        qT_dst = outqp.tile([P, 2, SC], F32R, tag="qTc")
                    ssl = slice(sc * SC, (sc + 1) * SC)
                    xt, cos_t, sin_t = loaded

                    for hl in range(2):
                        for t in range(2):  # 0=q, 1=k
                            wcols = slice(
                                (2 * hl + t) * HD, (2 * hl + t + 1) * HD
                            )
                            ps = cyc.tile([P, SC], F32, tag="ps")
                            for ko in range(KO):
                                nc.tensor.matmul(
                                    ps[:],
                                    w_sb[:, ko, wcols],
                                    xt[:, ko],
                                    start=(ko == 0),
                                    stop=(ko == KO - 1),
                                )
                            plain = scr.tile([P, SC], F32R, tag="plain")
                            nc.scalar.copy(plain[:], ps[:])
                            rot = cyc.tile([P, SC], F32, tag="ps")
                            nc.tensor.matmul(rot[:], pt_sb[:], plain[:])
                            if t == 0:
                                dst = (
                                    qT_dst[:, hl, ssl]
                                    if deferred
                                    else qT_dst[:, hl, :]
                                )
                            else:
                                dst = kT_sb[:, hl, ssl]
                            # rope: dst = plain*cos + rot*sin
                            # (final add on DVE performs the f32r rounding)
                            pc = scr.tile([P, SC], F32, tag="pc")
                            nc.gpsimd.tensor_mul(pc[:], plain[:], cos_t[:])
                            tmp2 = scr.tile([P, SC], F32, tag="tmp2")
                            nc.vector.tensor_mul(tmp2[:], rot[:], sin_t[:])
                            nc.vector.tensor_add(dst, pc[:], tmp2[:])
                            # (rope-mul engine choice patched in bisection)

                    for sti in range(NSUB):
                        st = sc * NSUB + sti
                        lsl = slice(sti * P, (sti + 1) * P)
                        psv = cyc.tile([P, 2 * HD], F32, tag="ps")
                        for ko in range(KO):
                            nc.tensor.matmul(
                                psv[:],
                                xt[:, ko, lsl],
                                w_sb[:, ko, 4 * HD : 6 * HD],
                                start=(ko == 0),
                                stop=(ko == KO - 1),
                            )
                        vdst = v_sb[:, st, :]
                        nc.scalar.copy(vdst, psv[:])
                    return qT_dst

                def attend_chunk(qc, qT_cur, outT_qc, fillers):
                    """Attention for query chunk qc, both heads of the pair
                    interleaved per k-block (so the PE always has two
                    independent softmax chains in flight), writing
                    normalized outT [hd, q] slices.  `fillers` is a deque
                    of independent PE-work closures drained into the
                    pipeline's tail bubbles."""
                    nkb = (qc + 1) * NSUB if causal else NST
                    qt = {}
                    o_ps = {}
                    d_ps = {}
                    for hl in range(2):
                        qt[hl] = (
                            qT_cur[:, hl, qc * SC : (qc + 1) * SC]
                            if deferred
                            else qT_cur[:, hl, :]
                        )
                        o_ps[hl] = ops.tile([P, SC], F32, tag="o", name=f"o_ps{hl}")
                        d_ps[hl] = dps.tile([P, SC], F32, tag="d", name=f"d_ps{hl}")
                    stile = {}

                    def emit_scores(kb, hl):
                        t_ = cyc.tile([P, SC], F32, tag="ps")
                        nc.tensor.matmul(
                            t_[:],
                            kT_sb[:, hl, kb * P : (kb + 1) * P],
                            qt[hl],
                            skip_group_check=True,
                        )
                        if causal:
                            j = kb - qc * NSUB
                            if j >= 0:
                                w_ = P * (j + 1)
                                nc.vector.tensor_add(
                                    t_[:, :w_], t_[:, :w_],
                                    mask_sb[:, j, :w_],
                                )
                        else:
                            if hl == 0:
                                mt = gmp.tile([P, SC], F32, tag="mt")
                                nc.sync.dma_start(
                                    mt[:],
                                    maskT[
                                        kb * P : (kb + 1) * P,
                                        qc * SC : (qc + 1) * SC,
                                    ],
                                )
                                stile[("m", kb)] = mt
                            nc.vector.tensor_add(
                                t_[:], t_[:], stile[("m", kb)][:]
                            )
                        stile[(kb, hl)] = t_

                    # lookahead in (kb, hl) steps; 3 PSUM slots for scores
                    seq = [(kb, hl) for kb in range(nkb) for hl in range(2)]
                    for kb, hl in seq[:LOOKAHEAD]:
                        emit_scores(kb, hl)
                    for i, (kb, hl) in enumerate(seq):
                        e = expp.tile([P, SC], F32R, tag="e")
                        nc.scalar.activation(
                            e[:], stile.pop((kb, hl))[:], AF.Exp,
                            scale=inv_sqrt_hd,
                        )
                        nc.tensor.matmul(
                            o_ps[hl][:],
                            v_sb[:, kb, hl * HD : (hl + 1) * HD],
                            e[:],
                            start=(kb == 0),
                            stop=(kb == nkb - 1),
                            skip_group_check=True,
                        )
                        nc.tensor.matmul(
                            d_ps[hl][:],
                            ones_sb[:],
                            e[:],
                            start=(kb == 0),
                            stop=(kb == nkb - 1),
                            skip_group_check=True,
                        )
                        if i + LOOKAHEAD < len(seq):
                            emit_scores(*seq[i + LOOKAHEAD])
                            if fillers and i % 2 == 1:
                                fillers.popleft()()
                        elif fillers:
                            fillers.popleft()()
                    for hl in range(2):
                        recip = scr.tile([P, SC], F32, tag="recip")
                        nc.vector.reciprocal_approx_fast(
                            recip[:], d_ps[hl][:]
                        )
                        nc.vector.tensor_mul(
                            outT_qc[:, hl, :], o_ps[hl][:], recip[:]
                        )

                def make_out_fillers(hb, qc, outT_qc):
                    """One closure per (s-tile, d-chunk) block of the wo
                    projection for query chunk qc: 2 accumulating matmuls,
                    a PSUM->SBUF copy (alternating DVE/ACT), and the output
                    DMA (plain HWDGE write for pair 0, CCE accumulate for
                    pair 1)."""
                    work = []
                    for sti in range(NSUB):
                        st = qc * NSUB + sti
                        stsl = slice(sti * P, (sti + 1) * P)
                        for dc in range(D // SC):
                            dsl = slice(dc * SC, (dc + 1) * SC)

                            def blk(st=st, stsl=stsl, dsl=dsl):
                                y_ps = yps.tile([P, SC], F32, tag="y")
                                for hl in range(2):
                                    nc.tensor.matmul(
                                        y_ps[:],
                                        outT_qc[:, hl, stsl],
                                        wo_sb[:, hl, dsl],
                                        start=(hl == 0),
                                        stop=(hl == 1),
                                    )
                                y_sb = yop.tile([P, SC], F32, tag="ysb")
                                nc.vector.tensor_copy(y_sb[:], y_ps[:])
                                ydst = y[st * P : (st + 1) * P, dsl]
                                if hb == 0:
                                    nc.sync.dma_start(ydst, y_sb[:])
                                else:
                                    nc.gpsimd.dma_start(
                                        ydst, y_sb[:],
                                        accum_op=mybir.AluOpType.add,
                                    )

                            work.append(blk)
                    return work

                pending = deque()
                if causal:
                    for sc in range(NQC):
                        loaded = preloaded if sc == 0 and preloaded else load_chunk(sc)
                        preloaded = None
                        qT_cur = project_chunk(sc, loaded)
                        outT_qc = outqp.tile([P, 2, SC], F32R, tag="outq")
                        attend_chunk(sc, qT_cur, outT_qc, pending)
                        pending.extend(make_out_fillers(hb, sc, outT_qc))
                else:
                    for sc in range(NQC):
                        loaded = preloaded if sc == 0 and preloaded else load_chunk(sc)
                        preloaded = None
                        project_chunk(sc, loaded)
                    for qc in range(NQC):
                        outT_qc = outqp.tile([P, 2, SC], F32R, tag="outq")
                        attend_chunk(qc, qT_full, outT_qc, pending)
                        pending.extend(make_out_fillers(hb, qc, outT_qc))
                while pending:
                    pending.popleft()()

    nc.compile()
    return nc


_NC_CACHE = {}


def _get_nc(causal: bool):
    if causal not in _NC_CACHE:
        _NC_CACHE[causal] = _build_core_kernel(causal)
    return _NC_CACHE[causal]


def _rope_perm_T() -> np.ndarray:
    # rotate_half as a matrix: (P_rh @ q)[d] = -q[d+HD/2] for d < HD/2,
    # q[d-HD/2] otherwise.  Returns P_rh.T for use as matmul lhsT.
    P_rh = np.zeros((HD, HD), dtype=np.float32)
    half = HD // 2
    for i in range(half):
        P_rh[i, half + i] = -1.0
        P_rh[half + i, i] = 1.0
    return np.ascontiguousarray(P_rh.T)


def _is_causal(m: np.ndarray) -> bool:
    tril = np.tril(np.ones((S, S), dtype=bool))
    if not np.all(m[tril] == 0.0):
        return False
    upper = m[~tril]
    return bool(upper.size == 0 or np.all(upper <= -1.0e8))


# module-level: results of the last traced run (for test harnesses)
last_exec_time_ns = None
last_profile_json = None


def kernel(x, cos, sin, mask, wq, wk, wv, wo, _trace=False):
    x = np.asarray(x, dtype=np.float32)
    cos = np.asarray(cos, dtype=np.float32)
    sin = np.asarray(sin, dtype=np.float32)
    mask = np.asarray(mask, dtype=np.float32)
    wq = np.asarray(wq, dtype=np.float32)
    wk = np.asarray(wk, dtype=np.float32)
    wv = np.asarray(wv, dtype=np.float32)
    wo = np.asarray(wo, dtype=np.float32)

    m2d = mask.reshape(S, S)
    causal = _is_causal(m2d)
    nc = _get_nc(causal)

    scale = np.float32(np.sqrt(HD))
    if causal:
        maskT = np.ascontiguousarray((m2d[:SC, :SC] * scale).T)
    else:
        maskT = np.ascontiguousarray((m2d * scale).T)
    cosT = np.ascontiguousarray(cos.T, dtype=np.float32)
    sinT = np.ascontiguousarray(sin.T, dtype=np.float32)
    ptT = _round_f32r(_rope_perm_T())
    ones = np.ones((P, P), dtype=np.float32)

    xT = [_round_f32r(x[b].T) for b in range(B)]

    in_maps = []
    for c in range(N_CORES):
        b = c // (N_CORES // B)
        hg = c % (N_CORES // B)
        rows = slice(hg * HW, (hg + 1) * HW)
        # pack per head-pair: [q_h0 | k_h0 | q_h1 | k_h1 | v_h0 | v_h1]
        packs = []
        for hbp in range(H_LOC // 2):
            cols = []
            for hl in range(2):
                h = hg * H_LOC + hbp * 2 + hl
                cols.append(wq[h * HD : (h + 1) * HD].T)
                cols.append(wk[h * HD : (h + 1) * HD].T)
            for hl in range(2):
                h = hg * H_LOC + hbp * 2 + hl
                cols.append(wv[h * HD : (h + 1) * HD].T)
            packs.append(np.concatenate(cols, axis=1))
        wqkvT = np.stack(packs)
        in_maps.append(
            {
                "xT": xT[b],
                "wqkvT": _round_f32r(wqkvT),
                "woT": _round_f32r(np.ascontiguousarray(wo[:, rows].T)),
                "cosT": cosT,
                "sinT": sinT,
                "PT": ptT,
                "ones": ones,
                "maskT": maskT.astype(np.float32),
            }
        )

    kw = {}
    if _trace:
        kw = dict(trace=True)
    res = run_bass_kernel_spmd(
        nc, in_maps, core_ids=list(range(N_CORES)), **kw
    )
    global last_exec_time_ns, last_profile_json
    last_exec_time_ns = res.exec_time_ns
    last_profile_json = res.profile_json

    out = np.empty((B, S, D), dtype=np.float32)
    gs = N_CORES // B
    for b in range(B):
        acc = res.results[b * gs]["y"].astype(np.float32).copy()
        for g in range(1, gs):
            acc += res.results[b * gs + g]["y"]
        out[b] = acc
    return out



# revision 6
# speedup vs baseline: 1.5428x; 1.5428x over previous
"""Trainium2 8-core kernel for nn_Attention_27530740367526.

Multi-head causal attention (B=2, S=2048, D=2048, H=16, HD=128) with RoPE,
sharded batch x head-group across 8 NeuronCores: core c handles batch c//4
and heads [4*(c%4), 4*(c%4)+4).  Each core computes q/k/v projections
(+RoPE), attention for its 4 heads, and its heads' slice of the wo
projection -- a partial [S, D] output.  The host sums the 4 partials per
batch (the row-parallel wo "all-reduce" is a host-side unshard).

All matmul operands are bf16 (PSUM accumulation is fp32), which runs at
full PE rate, halves DMA/SBUF traffic vs f32r, and keeps LDWEIGHTS cheap.
Everything lives in "transposed land": qT/kT are [head_dim, seq] with
head-dim on partitions, so scores come out transposed ([k, q]), the
softmax denominator is an all-ones-column matmul (partition-broadcast
denominator for free), and PV / wo consume natural layouts with zero
on-device transposes.  RoPE's rotate-half is a 128x128 permutation matmul.

Schedule per core (single pass over all 4 heads -- y is written once):
  P0 A0 P1 A1+W0 P2 A2+W1 P3 A3+W2 W3
where P(sc) projects q/k/v for 512-seq chunk sc (dense PE phase, next x
chunk prefetched via split DMA queues), A(qc) runs causal attention for
query chunk qc as two 2-head interleaved softmax chains, and W(qc) is the
wo projection of chunk qc cut into 16 [128,512] blocks used as PE filler
inside the NEXT attention phase's exp-wait bubbles (one 4-matmul block
between a step's exp and its PV keeps the PE continuously busy, which
also keeps the PE p-state clock at max).
"""

import sys

if "/opt/trn_rl_repo" not in sys.path:
    sys.path.insert(0, "/opt/trn_rl_repo")

from collections import deque

import ml_dtypes
import numpy as np

import concourse.bacc as bacc
import concourse.mybir as mybir
import concourse.tile as tile
from concourse.bass_utils import run_bass_kernel_spmd

F32 = mybir.dt.float32
BF16 = mybir.dt.bfloat16
AF = mybir.ActivationFunctionType
BF_NP = ml_dtypes.bfloat16

N_HEADS = 16
N_CORES = 8
B, S, D = 2, 2048, 2048
HD = D // N_HEADS
H_LOC = N_HEADS // (N_CORES // B)  # 4 heads per core
HW = H_LOC * HD                    # 512 wo rows per core
SC = 512                           # seq chunk (matmul moving free dim)
P = 128
KO = D // P                        # 16 contraction subtiles
NQC = S // SC                      # 4 q-chunks
NSUB = SC // P                     # 4 128-blocks per chunk
NST = S // P                       # 16 s-tiles
LA = 2                             # scores-tile software pipeline depth


def _build_core_kernel(causal: bool):
    inv_sqrt_hd = 1.0 / float(np.sqrt(HD))

    nc = bacc.Bacc(None, target_bir_lowering=False)

    xT = nc.dram_tensor("xT", [D, S], BF16, kind="ExternalInput")
    wqkvT = nc.dram_tensor("wqkvT", [D, 12 * HD], BF16, kind="ExternalInput")
    woT = nc.dram_tensor("woT", [HW, D], BF16, kind="ExternalInput")
    cosT = nc.dram_tensor("cosT", [HD, S], BF16, kind="ExternalInput")
    sinT = nc.dram_tensor("sinT", [HD, S], BF16, kind="ExternalInput")
    PT = nc.dram_tensor("PT", [HD, HD], BF16, kind="ExternalInput")
    ones = nc.dram_tensor("ones", [P, P], BF16, kind="ExternalInput")
    if causal:
        maskT = nc.dram_tensor("maskT", [SC, SC], F32, kind="ExternalInput")
    else:
        maskT = nc.dram_tensor("maskT", [S, S], F32, kind="ExternalInput")
    y = nc.dram_tensor("y", [S, D], BF16, kind="ExternalOutput")

    xT_r = xT.rearrange("(ko ki) s -> ki ko s", ki=P)
    wqkvT_r = wqkvT.rearrange("(ko ki) c -> ki ko c", ki=P)
    woT_r = woT.rearrange("(h ki) d -> ki h d", ki=P)

    with tile.TileContext(nc) as tc:
        with (
            tc.tile_pool(name="persist", bufs=1) as persist,
            tc.tile_pool(name="xa", bufs=2) as xa,
            tc.tile_pool(name="qp", bufs=2) as qpool,
            tc.tile_pool(name="op", bufs=2) as opool,
            tc.tile_pool(name="plainp", bufs=3) as plainp,
            tc.tile_pool(name="ropet", bufs=2) as ropet,
            tc.tile_pool(name="ep", bufs=5) as ep,
            tc.tile_pool(name="yo", bufs=3) as yop,
            tc.tile_pool(name="scr", bufs=2) as scrp,
            tc.tile_pool(name="gm", bufs=3) as gmp,
            tc.tile_pool(name="acc", bufs=4, space="PSUM") as accp,
            tc.tile_pool(name="sc2", bufs=LA, space="PSUM") as sc2,
            tc.tile_pool(name="y2", bufs=2, space="PSUM") as y2,
        ):
            # ---- initial DMAs.  x chunk 0 round-robins sync/vector so the
            # first projection chain can start within ~1 us; the qkv weights
            # stream per-(head,type) column block on gpsimd/scalar so chain
            # i's weights arrive before chain i needs them; wo + the v
            # weight block follow on the same queues (needed much later).
            xt0 = xa.tile([P, KO, SC], BF16, tag="xt", name="xt0")
            for ko in range(KO):
                eng = nc.sync if ko % 2 == 0 else nc.gpsimd
                eng.dma_start(xt0[:, ko], xT_r[:, ko, 0:SC])

            w_sb = persist.tile([P, KO, 12 * HD], BF16, tag="w", name="w_sb")
            for i in range(8):  # 8 q/k chains: (h, t) -> cols (2h+t)*HD
                nc.scalar.dma_start(
                    w_sb[:, :, i * HD : (i + 1) * HD],
                    wqkvT_r[:, :, i * HD : (i + 1) * HD],
                )
            pt_sb = persist.tile([P, HD], BF16, tag="pt", name="pt_sb")
            nc.sync.dma_start(pt_sb[:], PT[:])
            ones_sb = persist.tile([P, P], BF16, tag="ones", name="ones_sb")
            nc.sync.dma_start(ones_sb[:], ones[:])
            cos_sb = persist.tile([P, S], BF16, tag="cos", name="cos_sb")
            nc.sync.dma_start(cos_sb[:], cosT[:])
            sin_sb = persist.tile([P, S], BF16, tag="sin", name="sin_sb")
            nc.sync.dma_start(sin_sb[:], sinT[:])
            if causal:
                mask_sb = persist.tile([P, NSUB, SC], F32, tag="mask", name="mask_sb")
                nc.sync.dma_start(
                    mask_sb[:], maskT.rearrange("(j ki) q -> ki j q", ki=P)
                )
            # v weight block + wo (first needed ~25 us / ~60 us in)
            nc.scalar.dma_start(
                w_sb[:, :, 8 * HD : 12 * HD], wqkvT_r[:, :, 8 * HD : 12 * HD]
            )
            wo_sb = persist.tile([P, H_LOC, D], BF16, tag="wo", name="wo_sb")
            for h in range(H_LOC):
                nc.gpsimd.dma_start(wo_sb[:, h], woT_r[:, h, :])

            kT_sb = persist.tile([P, H_LOC, S], BF16, tag="kT", name="kT_sb")
            v_sb = persist.tile([P, NST, H_LOC * HD], BF16, tag="v", name="v_sb")
            qT_full = (
                persist.tile([P, H_LOC, S], BF16, tag="qTf", name="qT_full")
                if not causal
                else None
            )

            def load_chunk(sc):
                # prefetched a full phase ahead -> one big descriptor
                ssl = slice(sc * SC, (sc + 1) * SC)
                xt = xa.tile([P, KO, SC], BF16, tag="xt", name=f"xt{sc}")
                nc.sync.dma_start(xt[:], xT_r[:, :, ssl])
                return xt

            def project_chunk(sc, xt, qT_c):
                """q/k (+RoPE) and v projections for seq chunk sc.  The
                RoPE for chain i is emitted during chain i+1's matmuls so
                the rotate-half matmul never stalls the PE on the
                PSUM->SBUF copy."""
                ssl = slice(sc * SC, (sc + 1) * SC)
                pending_rope = []

                def flush_rope():
                    for h, t, plain, dst in pending_rope:
                        rot = sc2.tile([P, SC], F32, tag="sc", name="rot")
                        nc.tensor.matmul(rot[:], pt_sb[:], plain[:])
                        pc = ropet.tile([P, SC], F32, tag="pc", name="pc")
                        nc.gpsimd.tensor_mul(pc[:], plain[:], cos_sb[:, ssl])
                        t2 = ropet.tile([P, SC], F32, tag="t2", name="t2")
                        nc.vector.tensor_mul(t2[:], rot[:], sin_sb[:, ssl])
                        nc.vector.tensor_add(dst, pc[:], t2[:])
                    pending_rope.clear()

                for h in range(H_LOC):
                    for t in range(2):  # 0=q, 1=k
                        wcols = slice((2 * h + t) * HD, (2 * h + t + 1) * HD)
                        ps = accp.tile([P, SC], F32, tag="acc", name="ps")
                        for ko in range(KO):
                            nc.tensor.matmul(
                                ps[:],
                                w_sb[:, ko, wcols],
                                xt[:, ko],
                                start=(ko == 0),
                                stop=(ko == KO - 1),
                            )
                        plain = plainp.tile([P, SC], BF16, tag="plain", name="plain")
                        nc.scalar.copy(plain[:], ps[:])
                        if t == 0:
                            dst = qT_c[:, h, ssl] if qT_c is qT_full else qT_c[:, h, :]
                        else:
                            dst = kT_sb[:, h, ssl]
                        flush_rope()
                        pending_rope.append((h, t, plain, dst))

                for sti in range(NSUB):
                    st = sc * NSUB + sti
                    lsl = slice(sti * P, (sti + 1) * P)
                    psv = accp.tile([P, H_LOC * HD], F32, tag="acc", name="psv")
                    for ko in range(KO):
                        nc.tensor.matmul(
                            psv[:],
                            xt[:, ko, lsl],
                            w_sb[:, ko, 8 * HD : 12 * HD],
                            start=(ko == 0),
                            stop=(ko == KO - 1),
                        )
                    flush_rope()
                    nc.scalar.copy(v_sb[:, st, :], psv[:])

            def attend_half(qc, half, qT_c, outT_qc, fillers):
                """Attention for query chunk qc, heads (2*half, 2*half+1)
                interleaved per k-block.  One filler block (4 wo matmuls)
                is drained between a step's exp and its PV matmul so the
                PE bridges the exp latency with independent work."""
                nkb = (qc + 1) * NSUB if causal else NST
                hs = (2 * half, 2 * half + 1)
                qt = {}
                o_ps = {}
                d_ps = {}
                for hp in range(2):
                    qt[hp] = (
                        qT_c[:, hs[hp], qc * SC : (qc + 1) * SC]
                        if qT_c is qT_full
                        else qT_c[:, hs[hp], :]
                    )
                    o_ps[hp] = accp.tile([P, SC], F32, tag="acc", name=f"o{hp}")
                    d_ps[hp] = accp.tile([P, SC], F32, tag="acc", name=f"d{hp}")
                stile = {}

                def emit_scores(kb, hp):
                    t_ = sc2.tile([P, SC], F32, tag="sc", name="scores")
                    nc.tensor.matmul(
                        t_[:],
                        kT_sb[:, hs[hp], kb * P : (kb + 1) * P],
                        qt[hp],
                        skip_group_check=True,
                    )
                    if causal:
                        j = kb - qc * NSUB
                        if j >= 0:
                            w_ = P * (j + 1)
                            nc.vector.tensor_add(
                                t_[:, :w_], t_[:, :w_], mask_sb[:, j, :w_]
                            )
                    else:
                        if hp == 0:
                            mt = gmp.tile([P, SC], F32, tag="mt", name="mt")
                            nc.sync.dma_start(
                                mt[:],
                                maskT[
                                    kb * P : (kb + 1) * P,
                                    qc * SC : (qc + 1) * SC,
                                ],
                            )
                            stile[("m", kb)] = mt
                        nc.vector.tensor_add(t_[:], t_[:], stile[("m", kb)][:])
                    stile[(kb, hp)] = t_

                seq = [(kb, hp) for kb in range(nkb) for hp in range(2)]
                for s_ in seq[:LA]:
                    emit_scores(*s_)
                for i, (kb, hp) in enumerate(seq):
                    e = ep.tile([P, SC], BF16, tag="e", name="e")
                    nc.scalar.activation(
                        e[:], stile.pop((kb, hp))[:], AF.Exp, scale=inv_sqrt_hd
                    )
                    if fillers and i % 2 == 0:
                        fillers.popleft()()
                    h = hs[hp]
                    nc.tensor.matmul(
                        o_ps[hp][:],
                        v_sb[:, kb, h * HD : (h + 1) * HD],
                        e[:],
                        start=(kb == 0),
                        stop=(kb == nkb - 1),
                        skip_group_check=True,
                    )
                    nc.tensor.matmul(
                        d_ps[hp][:],
                        ones_sb[:],
                        e[:],
                        start=(kb == 0),
                        stop=(kb == nkb - 1),
                        skip_group_check=True,
                    )
                    if i + LA < len(seq):
                        emit_scores(*seq[i + LA])
                for hp in range(2):
                    recip = scrp.tile([P, SC], F32, tag="recip", name="recip")
                    nc.vector.reciprocal_approx_fast(recip[:], d_ps[hp][:])
                    nc.vector.tensor_mul(
                        outT_qc[:, hs[hp], :], o_ps[hp][:], recip[:]
                    )

            def make_wo_blocks(qc, outT_qc):
                """16 [128,512] wo-projection blocks for query chunk qc:
                4 accumulating matmuls (one per head), a PSUM->SBUF copy
                alternating DVE/ACT, and the y output DMA."""
                work = []
                for sti in range(NSUB):
                    st = qc * NSUB + sti
                    stsl = slice(sti * P, (sti + 1) * P)
                    row = {}
                    for dc in range(D // SC):
                        dsl = slice(dc * SC, (dc + 1) * SC)
                        bi = len(work)

                        def blk(st=st, stsl=stsl, dsl=dsl, bi=bi, dc=dc, row=row):
                            y_ps = y2.tile([P, SC], F32, tag="y", name="y_ps")
                            for h in range(H_LOC):
                                nc.tensor.matmul(
                                    y_ps[:],
                                    outT_qc[:, h, stsl],
                                    wo_sb[:, h, dsl],
                                    start=(h == 0),
                                    stop=(h == H_LOC - 1),
                                )
                            if dc == 0:
                                row["ysb"] = yop.tile(
                                    [P, D], BF16, tag="ysb", name="y_sb"
                                )
                            y_sb = row["ysb"]
                            if bi % 2 == 0:
                                nc.vector.tensor_copy(y_sb[:, dsl], y_ps[:])
                            else:
                                nc.scalar.copy(y_sb[:, dsl], y_ps[:])
                            if dc == D // SC - 1:
                                nc.sync.dma_start(
                                    y[st * P : (st + 1) * P, :], y_sb[:]
                                )

                        work.append(blk)
                return work

            pending = deque()
            if causal:
                xt_next = xt0
                for sc in range(NQC):
                    xt = xt_next
                    if sc + 1 < NQC:
                        xt_next = load_chunk(sc + 1)
                    qT_c = qpool.tile(
                        [P, H_LOC, SC], BF16, tag="qT", name=f"qT{sc}"
                    )
                    project_chunk(sc, xt, qT_c)
                    outT_qc = opool.tile(
                        [P, H_LOC, SC], BF16, tag="outT", name=f"outT{sc}"
                    )
                    attend_half(sc, 0, qT_c, outT_qc, pending)
                    attend_half(sc, 1, qT_c, outT_qc, pending)
                    pending.extend(make_wo_blocks(sc, outT_qc))
            else:
                xt_next = xt0
                for sc in range(NQC):
                    xt = xt_next
                    if sc + 1 < NQC:
                        xt_next = load_chunk(sc + 1)
                    project_chunk(sc, xt, qT_full)
                for qc in range(NQC):
                    outT_qc = opool.tile(
                        [P, H_LOC, SC], BF16, tag="outT", name=f"outT{qc}"
                    )
                    attend_half(qc, 0, qT_full, outT_qc, pending)
                    attend_half(qc, 1, qT_full, outT_qc, pending)
                    pending.extend(make_wo_blocks(qc, outT_qc))
            while pending:
                pending.popleft()()

    nc.compile()
    return nc


_NC_CACHE = {}


def _get_nc(causal: bool):
    if causal not in _NC_CACHE:
        _NC_CACHE[causal] = _build_core_kernel(causal)
    return _NC_CACHE[causal]


def _rope_perm_T() -> np.ndarray:
    # rotate_half as a matrix: (P_rh @ q)[d] = -q[d+HD/2] for d < HD/2,
    # q[d-HD/2] otherwise.  Returns P_rh.T for use as matmul lhsT.
    P_rh = np.zeros((HD, HD), dtype=np.float32)
    half = HD // 2
    for i in range(half):
        P_rh[i, half + i] = -1.0
        P_rh[half + i, i] = 1.0
    return np.ascontiguousarray(P_rh.T)


def _is_causal(m: np.ndarray) -> bool:
    tril = np.tril(np.ones((S, S), dtype=bool))
    if not np.all(m[tril] == 0.0):
        return False
    upper = m[~tril]
    return bool(upper.size == 0 or np.all(upper <= -1.0e8))


def _bf16(a: np.ndarray) -> np.ndarray:
    return np.ascontiguousarray(a, dtype=np.float32).astype(BF_NP)


# module-level: results of the last traced run (for test harnesses)
last_exec_time_ns = None
last_profile_json = None


def kernel(x, cos, sin, mask, wq, wk, wv, wo, _trace=False):
    x = np.asarray(x, dtype=np.float32)
    cos = np.asarray(cos, dtype=np.float32)
    sin = np.asarray(sin, dtype=np.float32)
    mask = np.asarray(mask, dtype=np.float32)
    wq = np.asarray(wq, dtype=np.float32)
    wk = np.asarray(wk, dtype=np.float32)
    wv = np.asarray(wv, dtype=np.float32)
    wo = np.asarray(wo, dtype=np.float32)

    m2d = mask.reshape(S, S)
    causal = _is_causal(m2d)
    nc = _get_nc(causal)

    scale = np.float32(np.sqrt(HD))
    if causal:
        maskT = np.ascontiguousarray((m2d[:SC, :SC] * scale).T)
    else:
        maskT = np.ascontiguousarray((m2d * scale).T)
    cosT = _bf16(cos.T)
    sinT = _bf16(sin.T)
    ptT = _bf16(_rope_perm_T())
    ones_m = np.ones((P, P), dtype=BF_NP)

    xT = [_bf16(x[b].T) for b in range(B)]

    in_maps = []
    for c in range(N_CORES):
        b = c // (N_CORES // B)
        hg = c % (N_CORES // B)
        rows = slice(hg * HW, (hg + 1) * HW)
        # pack cols: [q_h0|k_h0|q_h1|k_h1|q_h2|k_h2|q_h3|k_h3|v_h0..v_h3]
        cols = []
        for hl in range(H_LOC):
            h = hg * H_LOC + hl
            cols.append(wq[h * HD : (h + 1) * HD].T)
            cols.append(wk[h * HD : (h + 1) * HD].T)
        for hl in range(H_LOC):
            h = hg * H_LOC + hl
            cols.append(wv[h * HD : (h + 1) * HD].T)
        wqkvT = np.concatenate(cols, axis=1)
        in_maps.append(
            {
                "xT": xT[b],
                "wqkvT": _bf16(wqkvT),
                "woT": _bf16(np.ascontiguousarray(wo[:, rows].T)),
                "cosT": cosT,
                "sinT": sinT,
                "PT": ptT,
                "ones": ones_m,
                "maskT": maskT.astype(np.float32),
            }
        )

    kw = {}
    if _trace:
        kw = dict(trace=True)
    res = run_bass_kernel_spmd(
        nc, in_maps, core_ids=list(range(N_CORES)), **kw
    )
    global last_exec_time_ns, last_profile_json
    last_exec_time_ns = res.exec_time_ns
    last_profile_json = res.profile_json

    out = np.empty((B, S, D), dtype=np.float32)
    gs = N_CORES // B
    for b in range(B):
        acc = res.results[b * gs]["y"].astype(np.float32)
        for g in range(1, gs):
            acc += res.results[b * gs + g]["y"].astype(np.float32)
        out[b] = acc
    return out
